# revision 1
# baseline (speedup 1.0000x reference)
"""Multi-head causal attention (B=2, L=2048, D=1024, H=16) on 8 TRN2 cores.

Sharding: data-parallel over batch (cores 0-3 -> b=0, cores 4-7 -> b=1),
tensor-parallel over heads (each core computes 4 of the 16 heads: the
matching 256-column slice of Wq/Wk/Wv and 256-row slice of Wo).  Each core
returns a partial [L, D] output-projection contribution; the host sums the
4 partials per batch and adds bo.

Per-core kernel (all in transposed layout so the contraction dim is always
on SBUF partitions):
  - x^T via PE transposes, per 512-query window
  - Q^T/K^T = Wq/Wk^T x^T (+bias via DVE), V = x_val^T Wv, its bias folded
    into the PSUM->SBUF copy against a partition-replicated bv tile
  - S^T[k,q] per head with K=64 matmuls (even/odd head of a pair at
    partition base 0/64 -> concurrent PE row-groups)
  - P = exp(S/8) on ACT straight out of PSUM; causal zeroing of diagonal
    blocks post-exp via gpsimd affine_select (no max subtraction: scores
    are ~N(0,1), exp is overflow-safe in fp32)
  - O'^T = V_aug^T P^T with a ones column prepended to V so the softmax
    denominator accumulates in PSUM row 0
  - normalize is split: PSUM is evacuated immediately (frees the banks for
    the next head pair) and the pair's two denominator rows hop onto
    partitions 0/1 of one tile; the batched DVE reciprocal (free-dim-serial,
    so [2,512] costs the same as [1,512]), gpsimd partition broadcasts and
    DVE multiplies are deferred one window so they never precede the next
    window's PSUM->SBUF copies in the DVE's in-order stream; odd heads reach
    partitions 64..127 of O^T via one SWDGE SBUF->SBUF DMA per window
  - out_partial = O^T.T Wo; the previous window's output-projection tiles
    are interleaved into the attention loop as PE filler so the tensor
    engine stays busy (and the HAM clock gate stays warm) during exp waits
  - DMA queue roles: SP/HWDGE = input loads (+ output stores), gpsimd/SWDGE
    = small intra-SBUF moves, so big loads are never head-of-line blocked
    behind compute-gated transfers
"""

import numpy as np

import concourse.bass as bass
import concourse.tile as tile
from concourse import bacc, mybir
from concourse.bass_utils import run_bass_kernel_spmd
from concourse.vector_clock import VectorClock, ScopedClock

F32 = mybir.dt.float32
F32R = mybir.dt.float32r

B, L, D, H = 2, 2048, 1024, 16
DKH = 64          # head dim
HC = 4            # heads per core
DKC = HC * DKH    # 256 projected cols per core
LW = 512          # query window
NW = L // LW      # 4 windows
NKT = L // 128    # 16 k tiles
BF16 = mybir.dt.bfloat16
USE_BF16 = False
USE_F32R = True
MMDT = BF16 if USE_BF16 else (F32R if USE_F32R else F32)
TPDT = BF16 if USE_BF16 else F32R  # transpose path dtype


class _SplitDrainTileContext(tile.TileContext):
    """The walrus build in this container only supports a single sync-wait
    per Drain instruction; split the kernel-tail drain into one drain per
    outstanding semaphore."""

    def _drain_and_barrier(self, tick_clock, wait_clock):
        gc = tick_clock.global_clock
        n = len(gc)
        active = [i for i in range(n) if gc[i] > 0]
        for i in active:
            vc = VectorClock([gc[j] if j == i else 0 for j in range(n)])
            di = self.nc.sync.drain()
            wait_clock.add_sem_waits(di.ins, ScopedClock({None: vc}))
        self.nc.all_engine_barrier()
        popped = self.nc._tile_sem_poison_stack.pop()
        assert popped is self._sem_poison
        self.nc.clear_and_free_semaphores(list(self.sems.allocated().values()))
        self.nc.all_engine_barrier()


def _ms(ap):
    """memset-safe view: walrus rejects f32r memsets; bf16 is fine."""
    return ap.bitcast(F32) if ap.dtype == F32R else ap


def _wdma(nc, dst, src_ap):
    """DRAM f32 -> SBUF MMDT load: SWDGE cast for bf16, bit-identical
    HWDGE otherwise."""
    if USE_BF16:
        nc.gpsimd.dma_start(out=dst, in_=src_ap)
    else:
        nc.sync.dma_start(out=dst, in_=src_ap.bitcast(MMDT))


def build_program() -> bass.Bass:
    nc = bacc.Bacc("TRN2", target_bir_lowering=False, debug=False)

    x_ctx = nc.declare_dram_parameter("x_ctx", [L, D], F32, isOutput=False)
    x_val = nc.declare_dram_parameter("x_val", [L, D], F32, isOutput=False)
    wq = nc.declare_dram_parameter("wq", [D, DKC], F32, isOutput=False)
    wk = nc.declare_dram_parameter("wk", [D, DKC], F32, isOutput=False)
    wv = nc.declare_dram_parameter("wv", [D, DKC], F32, isOutput=False)
    bq = nc.declare_dram_parameter("bq", [DKC], F32, isOutput=False)
    bk = nc.declare_dram_parameter("bk", [DKC], F32, isOutput=False)
    bv = nc.declare_dram_parameter("bv", [DKC], F32, isOutput=False)
    wo = nc.declare_dram_parameter("wo", [DKC, D], F32, isOutput=False)
    ident_in = nc.declare_dram_parameter("ident", [128, 128], F32, isOutput=False)
    out = nc.declare_dram_parameter("out", [L, D], F32, isOutput=True)

    with _SplitDrainTileContext(nc) as tc:
        with (
            tc.tile_pool(name="consts", bufs=1) as consts,
            tc.tile_pool(name="resident", bufs=1) as resident,
            tc.tile_pool(name="xraw", bufs=2) as xraw_pool,
            tc.tile_pool(name="xT", bufs=2) as xT_pool,
            tc.tile_pool(name="qT", bufs=2) as qT_pool,
            tc.tile_pool(name="pp", bufs=3) as p_pool,
            tc.tile_pool(name="norm", bufs=2) as norm_pool,
            tc.tile_pool(name="evac", bufs=4) as evac_pool,
            tc.tile_pool(name="ps_mm", bufs=2, space="PSUM") as ps_mm,
            tc.tile_pool(name="ps_s", bufs=2, space="PSUM") as ps_s,
            tc.tile_pool(name="ps_o", bufs=2, space="PSUM") as ps_o,
        ):
            # ---- constants ----
            # identity comes from DRAM: gpsimd affine_select can't emit
            # f32r-rounded output, a bit-identical DMA can
            ident = consts.tile([128, 128], TPDT, tag="ident")
            nc.sync.dma_start(out=ident[:, :], in_=ident_in[:, :].bitcast(TPDT) if TPDT == F32R else ident_in[:, :])
            ones_b = consts.tile([128, 64], F32R, tag="onesb")
            nc.vector.memset(_ms(ones_b[:, :]), 1.0)
            # dummy activation: pulls the ~2.7us exp table load off the
            # first attention chunk's critical path
            warm = consts.tile([1, 8], F32, tag="warm")
            nc.vector.memset(warm[:, :], 0.0)
            nc.scalar.activation(warm[:, :], warm[:, :],
                                 func=mybir.ActivationFunctionType.Exp)

            xq = {}

            def load_window(src_dram, key, lw_):
                xw = xraw_pool.tile([128, 4, D], TPDT, tag="xraw")
                lsl_ = slice(lw_ * LW, (lw_ + 1) * LW)
                if USE_BF16:
                    # SWDGE cast-DMA: f32 HBM -> bf16 SBUF
                    nc.gpsimd.dma_start(out=xw[:, :, :],
                                        in_=src_dram[lsl_, :].rearrange("(a p) d -> p a d", p=128))
                else:
                    nc.sync.dma_start(out=xw[:, :, :],
                                      in_=src_dram[lsl_, :].rearrange("(a p) d -> p a d", p=128).bitcast(TPDT))
                xq[(key, lw_)] = xw

            # window-0 activations ahead of the 4MB of weights on the same
            # DMA queue, so the PE's first transposes start ~10us earlier
            load_window(x_ctx, 'c', 0)
            load_window(x_val, 'v', 0)

            wq_sb = consts.tile([128, 8, DKC], MMDT, tag="wq")
            _wdma(nc, wq_sb[:, :, :], wq[:, :].rearrange("(k p) n -> p k n", p=128))
            wk_sb = consts.tile([128, 8, DKC], MMDT, tag="wk")
            _wdma(nc, wk_sb[:, :, :], wk[:, :].rearrange("(k p) n -> p k n", p=128))
            wv_sb = consts.tile([128, 8, DKC], MMDT, tag="wv")
            _wdma(nc, wv_sb[:, :, :], wv[:, :].rearrange("(k p) n -> p k n", p=128))
            wo_sb = consts.tile([128, 2, D], MMDT, tag="wo")
            _wdma(nc, wo_sb[:, :, :], wo[:, :].rearrange("(m p) n -> p m n", p=128))

            bq_sb = consts.tile([128, 2], F32, tag="bq")
            nc.sync.dma_start(out=bq_sb[:, :], in_=bq[:].rearrange("(m p) -> p m", p=128))
            bk_sb = consts.tile([128, 2], F32, tag="bk")
            nc.sync.dma_start(out=bk_sb[:, :], in_=bk[:].rearrange("(m p) -> p m", p=128))
            # bv replicated to all 128 partitions via a step-0 DMA so the
            # V-projection bias folds into the PSUM->SBUF copy on the DVE
            bv_bc = consts.tile([128, DKC], F32, tag="bv")
            nc.sync.dma_start(
                out=bv_bc[:, :], in_=bv[:].unsqueeze(0).broadcast_to([128, DKC]))

            # ---- resident accumulators ----
            kT_sb = resident.tile([128, 2, L], MMDT, tag="kT")      # K^T, head h at [(h%2)*64:+64, h//2, :]
            v_sb = resident.tile([128, NKT, HC, 1 + DKH], MMDT, tag="v")  # V per l-tile/head: [V | ones]
            oT_sb = resident.tile([128, 2, L], MMDT, tag="oT")      # normalized O^T, same layout as kT

            nc.vector.memset(_ms(v_sb[:, :, :, DKH:DKH + 1]), 1.0)  # ones col -> denominator row

            def transpose_window(src_dram, key, lw_, xT):
                """PE-transpose a (possibly prefetched) window into xT."""
                if (key, lw_) not in xq:
                    load_window(src_dram, key, lw_)
                xw = xq.pop((key, lw_))
                for k in range(8):
                    pt = ps_mm.tile([128, LW], TPDT, tag="mm512")
                    for a in range(4):
                        nc.tensor.transpose(
                            pt[:, a * 128:(a + 1) * 128],
                            xw[:, a, k * 128:(k + 1) * 128], ident[:, :])
                    nc.vector.tensor_copy(xT[:, k, :], pt[:, :])

            pend = {}

            def finish_pair(lw_, onorm2, hp, ostg_e, ostg_o, rq):
                """Deferred half of one head pair's normalize: batched
                reciprocal, gpsimd broadcasts, DVE multiplies into O^T.
                No PE instructions, so it can never stall the PE."""
                lsl_ = slice(lw_ * LW, (lw_ + 1) * LW)
                rqr = norm_pool.tile([2, LW], F32, tag="rqr")
                nc.vector.reciprocal(rqr[:, :], rq[:, :])
                rr1 = norm_pool.tile([1, LW], F32, tag="rr1")
                nc.gpsimd.dma_start(out=rr1[0:1, :], in_=rqr[1:2, :])
                rb_e = norm_pool.tile([DKH, LW], F32, tag="rb")
                nc.gpsimd.partition_broadcast(rb_e[:, :], rqr[0:1, :])
                rb_o = norm_pool.tile([DKH, LW], F32, tag="rb")
                nc.gpsimd.partition_broadcast(rb_o[:, :], rr1[0:1, :])
                nc.vector.tensor_mul(
                    oT_sb[0:64, hp, lsl_], ostg_e[0:DKH, :], rb_e[:, :])
                nc.vector.tensor_mul(
                    onorm2[:, hp, :], ostg_o[0:DKH, :], rb_o[:, :])

            def finish_normalize(lw_):
                lsl_ = slice(lw_ * LW, (lw_ + 1) * LW)
                onorm2 = norm_pool.tile([DKH, 2, LW], MMDT, tag="onorm")
                for hp, ostg_e, ostg_o, rq in pend.pop(lw_):
                    finish_pair(lw_, onorm2, hp, ostg_e, ostg_o, rq)
                nc.gpsimd.dma_start(out=oT_sb[64:128, 0:2, lsl_], in_=onorm2[:, :, :])

            def out_proj_unit(lt, n):
                """One [128, 512] tile of partial = O^T.T @ Wo."""
                pop = ps_mm.tile([128, LW], F32, tag="mm512")
                for m in range(2):
                    nc.tensor.matmul(
                        pop[:, :],
                        oT_sb[:, m, lt * 128:(lt + 1) * 128],
                        wo_sb[:, m, n * 512:(n + 1) * 512],
                        start=(m == 0), stop=(m == 1),
                    )
                ost = p_pool.tile([128, LW], F32, tag="ostage")
                nc.vector.tensor_copy(ost[:, :], pop[:, :])
                nc.sync.dma_start(
                    out=out[lt * 128:(lt + 1) * 128, n * 512:(n + 1) * 512],
                    in_=ost[:, :],
                )

            for lw in range(NW):
                lsl = slice(lw * LW, (lw + 1) * LW)

                # ---- x_ctx window -> x^T; Q^T / K^T projections ----
                xT = xT_pool.tile([128, 8, LW], MMDT, tag="xT")
                transpose_window(x_ctx, 'c', lw, xT)

                qT = qT_pool.tile([128, 2, LW], MMDT, tag="qT")
                for m in range(2):
                    pq = ps_mm.tile([128, LW], F32, tag="mm512")
                    for k in range(8):
                        nc.tensor.matmul(
                            pq[:, :],
                            wq_sb[:, k, m * 128:(m + 1) * 128],
                            xT[:, k, :],
                            start=(k == 0), stop=(k == 7),
                        )
                    nc.vector.tensor_scalar_add(qT[:, m, :], pq[:, :], bq_sb[:, m:m + 1])
                    pk = ps_mm.tile([128, LW], F32, tag="mm512")
                    for k in range(8):
                        nc.tensor.matmul(
                            pk[:, :],
                            wk_sb[:, k, m * 128:(m + 1) * 128],
                            xT[:, k, :],
                            start=(k == 0), stop=(k == 7),
                        )
                    nc.vector.tensor_scalar_add(kT_sb[:, m, lsl], pk[:, :], bk_sb[:, m:m + 1])

                # ---- x_val window -> x^T -> V (native layout, +bias rank-1) ----
                xvT = xT_pool.tile([128, 8, LW], MMDT, tag="xT")
                transpose_window(x_val, 'v', lw, xvT)

                for a in range(4):
                    pv = ps_mm.tile([128, LW], F32, tag="mm512")
                    for k in range(8):
                        nc.tensor.matmul(
                            pv[:, 0:DKC],
                            xvT[:, k, a * 128:(a + 1) * 128],
                            wv_sb[:, k, :],
                            start=(k == 0), stop=(k == 7),
                        )
                    nc.vector.tensor_add(
                        v_sb[:, lw * 4 + a, :, 0:DKH],
                        pv[:, 0:DKC].rearrange("p (h d) -> p h d", h=HC),
                        bv_bc[:, :].rearrange("p (h d) -> p h d", h=HC),
                    )

                # ---- attention for this query window ----
                # prefetch next window's activations ahead of the sprinkled
                # output stores on the same DMA queue
                if lw + 1 < NW:
                    load_window(x_ctx, 'c', lw + 1)
                    load_window(x_val, 'v', lw + 1)

                nkt = 4 * (lw + 1)
                # previous window's output-projection units interleave into
                # this window's attention as PE filler during exp waits
                filler = ([(lt, n) for lt in range((lw - 1) * 4, lw * 4) for n in range(2)]
                          if lw > 0 else [])
                fi = 0
                if lw > 0:
                    finish_normalize(lw - 1)
                for hp in range(2):  # head pairs (2hp, 2hp+1)
                    po_e = ps_o.tile([1 + DKH, LW], F32, tag="o")
                    po_o = ps_o.tile([1 + DKH, LW], F32, tag="o")
                    for kt in range(nkt):
                        ksb = ps_s.tile([128, 2, LW], F32, tag="s")
                        nc.tensor.matmul(
                            ksb[:, 0, :],
                            kT_sb[0:64, hp, kt * 128:(kt + 1) * 128],
                            qT[0:64, hp, :],
                            start=True, stop=True,
                        )
                        nc.tensor.matmul(
                            ksb[:, 1, :],
                            kT_sb[64:128, hp, kt * 128:(kt + 1) * 128],
                            qT[64:128, hp, :],
                            start=True, stop=True,
                        )
                        psb = p_pool.tile([128, 2, LW], MMDT, tag="p")
                        nc.scalar.activation(
                            psb[:, :, :], ksb[:, :, :],
                            func=mybir.ActivationFunctionType.Exp,
                            scale=1.0 / np.sqrt(DKH),
                        )
                        s = kt - 4 * lw
                        if s >= 0:  # diagonal block: zero the upper-triangular part
                            for i in range(2):
                                nc.gpsimd.affine_select(
                                    out=psb[:, i, :], in_=psb[:, i, :],
                                    compare_op=mybir.AluOpType.is_ge,
                                    fill=0.0, base=-128 * s,
                                    pattern=[[1, LW]], channel_multiplier=-1,
                                )
                        nc.tensor.matmul(
                            po_e[:, :], v_sb[:, kt, 2 * hp, :], psb[:, 0, :],
                            start=(kt == 0), stop=(kt == nkt - 1),
                        )
                        nc.tensor.matmul(
                            po_o[:, :], v_sb[:, kt, 2 * hp + 1, :], psb[:, 1, :],
                            start=(kt == 0), stop=(kt == nkt - 1),
                        )
                        if fi < len(filler) and kt >= nkt - 4:
                            out_proj_unit(*filler[fi])
                            fi += 1
                    # evacuate both PSUM banks right away and hop the two
                    # denominator rows onto partitions 0/1 of one tile; the
                    # reciprocal + multiplies are deferred a window so they
                    # never precede the next window's PSUM->SBUF copies in
                    # the DVE's in-order stream
                    ostg_e = evac_pool.tile([1 + DKH, LW], F32, tag="ostg")
                    nc.vector.tensor_copy(ostg_e[:, :], po_e[:, :])
                    ostg_o = evac_pool.tile([1 + DKH, LW], F32, tag="ostg")
                    nc.vector.tensor_copy(ostg_o[:, :], po_o[:, :])
                    rq = norm_pool.tile([2, LW], F32, tag="rq")
                    nc.gpsimd.dma_start(out=rq[0:1, :], in_=ostg_e[64:65, :])
                    nc.gpsimd.dma_start(out=rq[1:2, :], in_=ostg_o[64:65, :])
                    if lw == NW - 1:
                        # no later window to defer into: finish right away so
                        # hp0's chain overlaps hp1's attention chunks
                        if hp == 0:
                            last_onorm2 = norm_pool.tile([DKH, 2, LW], MMDT, tag="onorm")
                        finish_pair(lw, last_onorm2, hp, ostg_e, ostg_o, rq)
                        if hp == 1:
                            nc.gpsimd.dma_start(
                                out=oT_sb[64:128, 0:2, lsl], in_=last_onorm2[:, :, :])
                    else:
                        pend.setdefault(lw, []).append((hp, ostg_e, ostg_o, rq))
                while fi < len(filler):
                    out_proj_unit(*filler[fi])
                    fi += 1

            for lt in range((NW - 1) * 4, NW * 4):
                for n in range(2):
                    out_proj_unit(lt, n)

    nc.compile()
    return nc


_CACHE = {}


def _program() -> bass.Bass:
    if "nc" not in _CACHE:
        _CACHE["nc"] = build_program()
    return _CACHE["nc"]


def make_in_maps(inputs):
    ctx = np.ascontiguousarray(np.asarray(inputs["context_sequence"], np.float32))
    val = np.ascontiguousarray(np.asarray(inputs["value_sequence"], np.float32))
    Wq = np.asarray(inputs["Wq"], np.float32)
    Wk = np.asarray(inputs["Wk"], np.float32)
    Wv = np.asarray(inputs["Wv"], np.float32)
    Wo = np.asarray(inputs["Wo"], np.float32)
    bq = np.asarray(inputs["bq"], np.float32)
    bk = np.asarray(inputs["bk"], np.float32)
    bv = np.asarray(inputs["bv"], np.float32)
    in_maps = []
    for c in range(8):
        b, hg = divmod(c, 4)
        cols = slice(hg * DKC, (hg + 1) * DKC)
        in_maps.append({
            "x_ctx": ctx[b],
            "x_val": val[b],
            "wq": np.ascontiguousarray(Wq[:, cols]),
            "wk": np.ascontiguousarray(Wk[:, cols]),
            "wv": np.ascontiguousarray(Wv[:, cols]),
            "bq": np.ascontiguousarray(bq[cols]),
            "bk": np.ascontiguousarray(bk[cols]),
            "bv": np.ascontiguousarray(bv[cols]),
            "wo": np.ascontiguousarray(Wo[cols, :]),
            "ident": np.eye(128, dtype=np.float32),
        })
    return in_maps


def combine_outputs(results, bo):
    bo = np.asarray(bo, np.float32)
    outs = [np.asarray(r["out"], np.float32) for r in results]
    full = np.empty((B, L, D), np.float32)
    for b in range(B):
        acc = np.zeros((L, D), np.float64)
        for c in range(4 * b, 4 * b + 4):
            acc += outs[c]
        full[b] = (acc + bo).astype(np.float32)
    return full


def _numpy_fallback(inputs):
    """Reference semantics for a non-causal mask (the TRN kernel hardcodes
    the causal structure)."""
    ctx = np.asarray(inputs["context_sequence"], np.float32)
    val = np.asarray(inputs["value_sequence"], np.float32)
    mask = np.asarray(inputs["mask"]) != 0
    Q = (ctx @ inputs["Wq"] + inputs["bq"]).reshape(B, L, H, DKH)
    Kp = (ctx @ inputs["Wk"] + inputs["bk"]).reshape(B, L, H, DKH)
    V = (val @ inputs["Wv"] + inputs["bv"]).reshape(B, L, H, DKH)
    outs = np.zeros((B, L, D), np.float32)
    for b in range(B):
        for h in range(H):
            s = (Q[b, :, h, :] @ Kp[b, :, h, :].T) / np.sqrt(np.float32(DKH))
            s = np.where(mask, s, -np.inf)
            s = s - s.max(axis=1, keepdims=True)
            p = np.exp(s)
            p /= p.sum(axis=1, keepdims=True)
            outs[b] += (p @ V[b, :, h, :]) @ np.asarray(inputs["Wo"])[h * DKH:(h + 1) * DKH, :]
    return outs + np.asarray(inputs["bo"], np.float32)


def kernel(**inputs) -> np.ndarray:
    mask = np.asarray(inputs["mask"])
    if not np.array_equal(mask != 0, np.tril(np.ones((L, L), bool))):
        return _numpy_fallback(inputs)
    nc = _program()
    in_maps = make_in_maps(inputs)
    last_err = None
    for _attempt in range(3):
        try:
            res = run_bass_kernel_spmd(nc, in_maps, list(range(8)))
            break
        except Exception as e:  # transient NRT device wedges clear on retry
            last_err = e
    else:
        raise last_err
    return combine_outputs(res.results, inputs["bo"])


if __name__ == "__main__":
    rng = np.random.default_rng(0)
    demo = {
        "context_sequence": rng.normal(size=(B, L, D)).astype(np.float32),
        "value_sequence": rng.normal(size=(B, L, D)).astype(np.float32),
        "mask": np.tril(np.ones((L, L), np.int32)),
        **{f"W{n}": (rng.normal(size=(D, D)) / 32).astype(np.float32) for n in "qkvo"},
        **{f"b{n}": (rng.normal(size=(D,)) / 32).astype(np.float32) for n in "qkvo"},
    }
    out = kernel(**demo)
    print(out.shape, out.dtype)



# revision 2
# speedup vs baseline: 1.2126x; 1.2126x over previous
"""Multi-head causal attention (B=2, L=2048, D=1024, H=16) on 8 TRN2 cores.

Sharding: data-parallel over batch (cores 0-3 -> b=0, cores 4-7 -> b=1),
tensor-parallel over heads (each core computes 4 of the 16 heads: the
matching 256-column slice of Wq/Wk/Wv and 256-row slice of Wo).  Each core
returns a partial [L, D] output-projection contribution; the host sums the
4 partials per batch and adds bo.

v2 design (vs the transpose-on-device baseline):
  - the host uploads x^T (and all weights) pre-transposed and pre-cast to
    bf16, so the kernel has no PE transposes and no PSUM->SBUF cast copies;
    every matmul operand is bf16 (full-rate PE, half the HBM traffic)
  - Phase A projects Q^T/K^T/V for ALL four 512-query windows first (the
    PE work overlaps the streaming x^T loads), then attention runs windows
    in order 3,2,1,0 so the largest window's output projection becomes
    PE filler for later windows and the kernel tail is the smallest window
  - attention inner loop is software-pipelined one k-tile ahead (S(kt+1)
    is issued before PV(kt)) so the exp on the scalar engine never stalls
    the PE
  - causal diagonal blocks restrict the matmul/exp/affine_select ranges to
    the valid triangle quarter instead of computing the full block
  - softmax denominators use reciprocal_approx_fast (~51 ULP, 5x faster
    than the bit-exact iterative divide)
"""

import numpy as np

import concourse.bass as bass
import concourse.tile as tile
from concourse import bacc, mybir
from concourse.bass_utils import run_bass_kernel_spmd
from concourse.vector_clock import VectorClock, ScopedClock

F32 = mybir.dt.float32
BF16 = mybir.dt.bfloat16

B, L, D, H = 2, 2048, 1024, 16
DKH = 64          # head dim
HC = 4            # heads per core
DKC = HC * DKH    # 256 projected cols per core
LW = 512          # query window
NW = L // LW      # 4 windows
NKT = L // 128    # 16 k tiles


class _SplitDrainTileContext(tile.TileContext):
    """The walrus build in this container only supports a single sync-wait
    per Drain instruction; split the kernel-tail drain into one drain per
    outstanding semaphore."""

    def _drain_and_barrier(self, tick_clock, wait_clock):
        gc = tick_clock.global_clock
        n = len(gc)
        active = [i for i in range(n) if gc[i] > 0]
        for i in active:
            vc = VectorClock([gc[j] if j == i else 0 for j in range(n)])
            di = self.nc.sync.drain()
            wait_clock.add_sem_waits(di.ins, ScopedClock({None: vc}))
        self.nc.all_engine_barrier()
        popped = self.nc._tile_sem_poison_stack.pop()
        assert popped is self._sem_poison
        self.nc.clear_and_free_semaphores(list(self.sems.allocated().values()))
        self.nc.all_engine_barrier()


def build_program() -> bass.Bass:
    nc = bacc.Bacc("TRN2", target_bir_lowering=False, debug=False)

    # x^T streams and weights arrive pre-transposed / pre-cast on the host
    xt_ctx = nc.declare_dram_parameter("xt_ctx", [D, L], BF16, isOutput=False)
    xt_val = nc.declare_dram_parameter("xt_val", [D, L], BF16, isOutput=False)
    wq = nc.declare_dram_parameter("wq", [D, DKC], BF16, isOutput=False)
    wk = nc.declare_dram_parameter("wk", [D, DKC], BF16, isOutput=False)
    wv = nc.declare_dram_parameter("wv", [D, DKC], BF16, isOutput=False)
    bq = nc.declare_dram_parameter("bq", [DKC], F32, isOutput=False)
    bk = nc.declare_dram_parameter("bk", [DKC], F32, isOutput=False)
    bv = nc.declare_dram_parameter("bv", [DKC], F32, isOutput=False)
    wo = nc.declare_dram_parameter("wo", [DKC, D], BF16, isOutput=False)
    out = nc.declare_dram_parameter("out", [L, D], F32, isOutput=True)

    with _SplitDrainTileContext(nc) as tc:
        with (
            tc.tile_pool(name="consts", bufs=1) as consts,
            tc.tile_pool(name="resident", bufs=1) as resident,
            tc.tile_pool(name="pp", bufs=3) as p_pool,
            tc.tile_pool(name="norm", bufs=2) as norm_pool,
            tc.tile_pool(name="evac", bufs=4) as evac_pool,
            tc.tile_pool(name="ps_mm", bufs=2, space="PSUM") as ps_mm,
            tc.tile_pool(name="ps_s", bufs=2, space="PSUM") as ps_s,
            tc.tile_pool(name="ps_o", bufs=2, space="PSUM") as ps_o,
        ):
            # dummy activation: pulls the ~2.7us exp table load off the
            # first attention chunk's critical path
            warm = consts.tile([1, 8], F32, tag="warm")
            nc.vector.memset(warm[:, :], 0.0)
            nc.scalar.activation(warm[:, :], warm[:, :],
                                 func=mybir.ActivationFunctionType.Exp)

            # ---- weights + per-window x^T loads, ordered so the first
            # projection can start after ~1.5MB ----
            wk_sb = consts.tile([128, 8, DKC], BF16, tag="wk")
            nc.sync.dma_start(out=wk_sb[:, :, :], in_=wk[:, :].rearrange("(k p) n -> p k n", p=128))
            bk_sb = consts.tile([128, 2], F32, tag="bk")
            nc.sync.dma_start(out=bk_sb[:, :], in_=bk[:].rearrange("(m p) -> p m", p=128))

            xc_sb = resident.tile([128, 8, L], BF16, tag="xc")
            xv_sb = resident.tile([128, 8, L], BF16, tag="xv")

            def load_x(lw_):
                lsl_ = slice(lw_ * LW, (lw_ + 1) * LW)
                nc.sync.dma_start(out=xc_sb[:, :, lsl_],
                                  in_=xt_ctx[:, lsl_].rearrange("(k p) l -> p k l", p=128))
                nc.sync.dma_start(out=xv_sb[:, :, lsl_],
                                  in_=xt_val[:, lsl_].rearrange("(k p) l -> p k l", p=128))

            load_x(0)

            wq_sb = consts.tile([128, 8, DKC], BF16, tag="wq")
            nc.sync.dma_start(out=wq_sb[:, :, :], in_=wq[:, :].rearrange("(k p) n -> p k n", p=128))
            bq_sb = consts.tile([128, 2], F32, tag="bq")
            nc.sync.dma_start(out=bq_sb[:, :], in_=bq[:].rearrange("(m p) -> p m", p=128))
            wv_sb = consts.tile([128, 8, DKC], BF16, tag="wv")
            nc.sync.dma_start(out=wv_sb[:, :, :], in_=wv[:, :].rearrange("(k p) n -> p k n", p=128))
            # bv replicated to all 128 partitions so the V-projection bias
            # folds into the PSUM->SBUF copy on the DVE
            bv_bc = consts.tile([128, DKC], F32, tag="bv")
            nc.sync.dma_start(
                out=bv_bc[:, :], in_=bv[:].unsqueeze(0).broadcast_to([128, DKC]))

            for lw_ in range(1, NW):
                load_x(lw_)

            wo_sb = consts.tile([128, 2, D], BF16, tag="wo")
            nc.sync.dma_start(out=wo_sb[:, :, :], in_=wo[:, :].rearrange("(m p) n -> p m n", p=128))

            # ---- resident projections ----
            qT_sb = resident.tile([128, 2, L], BF16, tag="qT")      # Q^T, head h at [(h%2)*64:+64, h//2, :]
            kT_sb = resident.tile([128, 2, L], BF16, tag="kT")      # K^T, same layout
            v_sb = resident.tile([128, NKT, HC, 1 + DKH], BF16, tag="v")  # V per l-tile/head: [V | ones]
            oT_sb = resident.tile([128, 2, L], BF16, tag="oT")      # normalized O^T, same layout as kT

            nc.vector.memset(v_sb[:, :, :, DKH:DKH + 1], 1.0)  # ones col -> denominator row

            # ---- Phase A: project K^T, Q^T, V for every window ----
            for lw in range(NW):
                lsl = slice(lw * LW, (lw + 1) * LW)
                for m in range(2):
                    pk = ps_mm.tile([128, LW], F32, tag="mm512")
                    for k in range(8):
                        nc.tensor.matmul(
                            pk[:, :],
                            wk_sb[:, k, m * 128:(m + 1) * 128],
                            xc_sb[:, k, lsl],
                            start=(k == 0), stop=(k == 7),
                        )
                    nc.vector.tensor_scalar_add(kT_sb[:, m, lsl], pk[:, :], bk_sb[:, m:m + 1])
                    pq = ps_mm.tile([128, LW], F32, tag="mm512")
                    for k in range(8):
                        nc.tensor.matmul(
                            pq[:, :],
                            wq_sb[:, k, m * 128:(m + 1) * 128],
                            xc_sb[:, k, lsl],
                            start=(k == 0), stop=(k == 7),
                        )
                    nc.vector.tensor_scalar_add(qT_sb[:, m, lsl], pq[:, :], bq_sb[:, m:m + 1])
                for a in range(4):
                    pv = ps_mm.tile([128, LW], F32, tag="mm512")
                    for k in range(8):
                        nc.tensor.matmul(
                            pv[:, 0:DKC],
                            xv_sb[:, k, lw * LW + a * 128:lw * LW + (a + 1) * 128],
                            wv_sb[:, k, :],
                            start=(k == 0), stop=(k == 7),
                        )
                    nc.vector.tensor_add(
                        v_sb[:, lw * 4 + a, :, 0:DKH],
                        pv[:, 0:DKC].rearrange("p (h d) -> p h d", h=HC),
                        bv_bc[:, :].rearrange("p (h d) -> p h d", h=HC),
                    )

            # ---- attention, windows largest-first so the tail is small ----
            pend = {}

            def finish_pair(lw_, onorm2, hp, ostg_e, ostg_o, rq):
                """Deferred half of one head pair's normalize: approx
                reciprocal, gpsimd broadcasts, DVE multiplies into O^T."""
                lsl_ = slice(lw_ * LW, (lw_ + 1) * LW)
                rqr = norm_pool.tile([2, LW], F32, tag="rqr")
                nc.vector.reciprocal_approx_fast(rqr[:, :], rq[:, :])
                rr1 = norm_pool.tile([1, LW], F32, tag="rr1")
                nc.gpsimd.dma_start(out=rr1[0:1, :], in_=rqr[1:2, :])
                rb_e = norm_pool.tile([DKH, LW], F32, tag="rb")
                nc.gpsimd.partition_broadcast(rb_e[:, :], rqr[0:1, :])
                rb_o = norm_pool.tile([DKH, LW], F32, tag="rb")
                nc.gpsimd.partition_broadcast(rb_o[:, :], rr1[0:1, :])
                nc.vector.tensor_mul(
                    oT_sb[0:64, hp, lsl_], ostg_e[0:DKH, :], rb_e[:, :])
                nc.vector.tensor_mul(
                    onorm2[:, hp, :], ostg_o[0:DKH, :], rb_o[:, :])

            def finish_normalize(lw_):
                lsl_ = slice(lw_ * LW, (lw_ + 1) * LW)
                onorm2 = norm_pool.tile([DKH, 2, LW], BF16, tag="onorm")
                for hp, ostg_e, ostg_o, rq in pend.pop(lw_):
                    finish_pair(lw_, onorm2, hp, ostg_e, ostg_o, rq)
                nc.gpsimd.dma_start(out=oT_sb[64:128, 0:2, lsl_], in_=onorm2[:, :, :])

            def out_proj_unit(lt, n):
                """One [128, 512] tile of partial = O^T.T @ Wo."""
                pop = ps_mm.tile([128, LW], F32, tag="mm512")
                for m in range(2):
                    nc.tensor.matmul(
                        pop[:, :],
                        oT_sb[:, m, lt * 128:(lt + 1) * 128],
                        wo_sb[:, m, n * 512:(n + 1) * 512],
                        start=(m == 0), stop=(m == 1),
                    )
                ost = p_pool.tile([128, LW], F32, tag="ostage")
                nc.vector.tensor_copy(ost[:, :], pop[:, :])
                nc.sync.dma_start(
                    out=out[lt * 128:(lt + 1) * 128, n * 512:(n + 1) * 512],
                    in_=ost[:, :],
                )

            order = list(range(NW - 1, -1, -1))  # 3, 2, 1, 0
            for wi, lw in enumerate(order):
                lsl = slice(lw * LW, (lw + 1) * LW)
                nkt = 4 * (lw + 1)
                prev = order[wi - 1] if wi > 0 else None
                if prev is not None:
                    finish_normalize(prev)
                # previous (larger) window's output projection interleaves
                # into this window's attention as PE filler
                filler = ([(lt, n) for lt in range(prev * 4, prev * 4 + 4) for n in range(2)]
                          if prev is not None else [])
                fi = 0
                for hp in range(2):  # head pairs (2hp, 2hp+1)
                    po_e = ps_o.tile([1 + DKH, LW], F32, tag="o")
                    po_o = ps_o.tile([1 + DKH, LW], F32, tag="o")
                    psbs = {}

                    def emit_S(kt):
                        s = kt - 4 * lw
                        qr = slice(128 * s, LW) if s > 0 else slice(0, LW)
                        ksb = ps_s.tile([128, 2, LW], F32, tag="s")
                        nc.tensor.matmul(
                            ksb[:, 0, qr],
                            kT_sb[0:64, hp, kt * 128:(kt + 1) * 128],
                            qT_sb[0:64, hp, lw * LW + qr.start:lw * LW + LW],
                            start=True, stop=True,
                        )
                        nc.tensor.matmul(
                            ksb[:, 1, qr],
                            kT_sb[64:128, hp, kt * 128:(kt + 1) * 128],
                            qT_sb[64:128, hp, lw * LW + qr.start:lw * LW + LW],
                            start=True, stop=True,
                        )
                        psb = p_pool.tile([128, 2, LW], BF16, tag="p")
                        nc.scalar.activation(
                            psb[:, :, qr], ksb[:, :, qr],
                            func=mybir.ActivationFunctionType.Exp,
                            scale=1.0 / np.sqrt(DKH),
                        )
                        if s >= 0:  # diagonal block: zero q<k in its 128-col strip
                            qs = slice(128 * s, 128 * s + 128)
                            for i in range(2):
                                nc.gpsimd.affine_select(
                                    out=psb[:, i, qs], in_=psb[:, i, qs],
                                    compare_op=mybir.AluOpType.is_ge,
                                    fill=0.0, base=0,
                                    pattern=[[1, 128]], channel_multiplier=-1,
                                )
                        psbs[kt] = psb

                    emit_S(0)
                    for kt in range(nkt):
                        if kt + 1 < nkt:
                            emit_S(kt + 1)
                        s = kt - 4 * lw
                        qr = slice(128 * s, LW) if s > 0 else slice(0, LW)
                        psb = psbs.pop(kt)
                        nc.tensor.matmul(
                            po_e[:, qr], v_sb[:, kt, 2 * hp, :], psb[:, 0, qr],
                            start=(kt == 0), stop=(kt == nkt - 1),
                        )
                        nc.tensor.matmul(
                            po_o[:, qr], v_sb[:, kt, 2 * hp + 1, :], psb[:, 1, qr],
                            start=(kt == 0), stop=(kt == nkt - 1),
                        )
                        if fi < len(filler) and kt >= nkt - 4:
                            out_proj_unit(*filler[fi])
                            fi += 1
                    # evacuate both PSUM banks right away; the denominator
                    # rows hop onto partitions 0/1 of one tile; the
                    # reciprocal + multiplies are deferred a window so they
                    # never precede the next window's PSUM->SBUF copies in
                    # the DVE's in-order stream
                    ostg_e = evac_pool.tile([1 + DKH, LW], F32, tag="ostg")
                    nc.vector.tensor_copy(ostg_e[:, :], po_e[:, :])
                    ostg_o = evac_pool.tile([1 + DKH, LW], F32, tag="ostg")
                    nc.vector.tensor_copy(ostg_o[:, :], po_o[:, :])
                    rq = norm_pool.tile([2, LW], F32, tag="rq")
                    nc.gpsimd.dma_start(out=rq[0:1, :], in_=ostg_e[64:65, :])
                    nc.gpsimd.dma_start(out=rq[1:2, :], in_=ostg_o[64:65, :])
                    if wi == NW - 1:
                        # last processed window: finish right away so hp0's
                        # chain overlaps hp1's attention chunks
                        if hp == 0:
                            last_onorm2 = norm_pool.tile([DKH, 2, LW], BF16, tag="onorm")
                        finish_pair(lw, last_onorm2, hp, ostg_e, ostg_o, rq)
                        if hp == 1:
                            nc.gpsimd.dma_start(
                                out=oT_sb[64:128, 0:2, lsl], in_=last_onorm2[:, :, :])
                    else:
                        pend.setdefault(lw, []).append((hp, ostg_e, ostg_o, rq))
                while fi < len(filler):
                    out_proj_unit(*filler[fi])
                    fi += 1

            last = order[-1]
            for lt in range(last * 4, last * 4 + 4):
                for n in range(2):
                    out_proj_unit(lt, n)

    nc.compile()
    return nc


_CACHE = {}


def _program() -> bass.Bass:
    if "nc" not in _CACHE:
        _CACHE["nc"] = build_program()
    return _CACHE["nc"]


def make_in_maps(inputs):
    import ml_dtypes
    bf16 = ml_dtypes.bfloat16
    ctx = np.asarray(inputs["context_sequence"], np.float32)
    val = np.asarray(inputs["value_sequence"], np.float32)
    Wq = np.asarray(inputs["Wq"], np.float32)
    Wk = np.asarray(inputs["Wk"], np.float32)
    Wv = np.asarray(inputs["Wv"], np.float32)
    Wo = np.asarray(inputs["Wo"], np.float32)
    bq = np.asarray(inputs["bq"], np.float32)
    bk = np.asarray(inputs["bk"], np.float32)
    bv = np.asarray(inputs["bv"], np.float32)
    xt_ctx = [np.ascontiguousarray(ctx[b].T).astype(bf16) for b in range(B)]
    xt_val = [np.ascontiguousarray(val[b].T).astype(bf16) for b in range(B)]
    in_maps = []
    for c in range(8):
        b, hg = divmod(c, 4)
        cols = slice(hg * DKC, (hg + 1) * DKC)
        in_maps.append({
            "xt_ctx": xt_ctx[b],
            "xt_val": xt_val[b],
            "wq": np.ascontiguousarray(Wq[:, cols]).astype(bf16),
            "wk": np.ascontiguousarray(Wk[:, cols]).astype(bf16),
            "wv": np.ascontiguousarray(Wv[:, cols]).astype(bf16),
            "bq": np.ascontiguousarray(bq[cols]),
            "bk": np.ascontiguousarray(bk[cols]),
            "bv": np.ascontiguousarray(bv[cols]),
            "wo": np.ascontiguousarray(Wo[cols, :]).astype(bf16),
        })
    return in_maps


def combine_outputs(results, bo):
    bo = np.asarray(bo, np.float32)
    outs = [np.asarray(r["out"], np.float32) for r in results]
    full = np.empty((B, L, D), np.float32)
    for b in range(B):
        acc = np.zeros((L, D), np.float64)
        for c in range(4 * b, 4 * b + 4):
            acc += outs[c]
        full[b] = (acc + bo).astype(np.float32)
    return full


def _numpy_fallback(inputs):
    """Reference semantics for a non-causal mask (the TRN kernel hardcodes
    the causal structure)."""
    ctx = np.asarray(inputs["context_sequence"], np.float32)
    val = np.asarray(inputs["value_sequence"], np.float32)
    mask = np.asarray(inputs["mask"]) != 0
    Q = (ctx @ inputs["Wq"] + inputs["bq"]).reshape(B, L, H, DKH)
    Kp = (ctx @ inputs["Wk"] + inputs["bk"]).reshape(B, L, H, DKH)
    V = (val @ inputs["Wv"] + inputs["bv"]).reshape(B, L, H, DKH)
    outs = np.zeros((B, L, D), np.float32)
    for b in range(B):
        for h in range(H):
            s = (Q[b, :, h, :] @ Kp[b, :, h, :].T) / np.sqrt(np.float32(DKH))
            s = np.where(mask, s, -np.inf)
            s = s - s.max(axis=1, keepdims=True)
            p = np.exp(s)
            p /= p.sum(axis=1, keepdims=True)
            outs[b] += (p @ V[b, :, h, :]) @ np.asarray(inputs["Wo"])[h * DKH:(h + 1) * DKH, :]
    return outs + np.asarray(inputs["bo"], np.float32)


def kernel(**inputs) -> np.ndarray:
    mask = np.asarray(inputs["mask"])
    if not np.array_equal(mask != 0, np.tril(np.ones((L, L), bool))):
        return _numpy_fallback(inputs)
    nc = _program()
    in_maps = make_in_maps(inputs)
    last_err = None
    for _attempt in range(3):
        try:
            res = run_bass_kernel_spmd(nc, in_maps, list(range(8)))
            break
        except Exception as e:  # transient NRT device wedges clear on retry
            last_err = e
    else:
        raise last_err
    return combine_outputs(res.results, inputs["bo"])


if __name__ == "__main__":
    rng = np.random.default_rng(0)
    demo = {
        "context_sequence": rng.normal(size=(B, L, D)).astype(np.float32),
        "value_sequence": rng.normal(size=(B, L, D)).astype(np.float32),
        "mask": np.tril(np.ones((L, L), np.int32)),
        **{f"W{n}": (rng.normal(size=(D, D)) / 32).astype(np.float32) for n in "qkvo"},
        **{f"b{n}": (rng.normal(size=(D,)) / 32).astype(np.float32) for n in "qkvo"},
    }
    out = kernel(**demo)
    print(out.shape, out.dtype)


# revision 13
# speedup vs baseline: 1.3973x; 1.1523x over previous
"""Multi-head causal attention (B=2, L=2048, D=1024, H=16) on 8 TRN2 cores.

Sharding: data-parallel over batch (cores 0-3 -> b=0, cores 4-7 -> b=1),
tensor-parallel over heads (each core computes 4 of the 16 heads: the
matching 256-column slice of Wq/Wk/Wv and 256-row slice of Wo).  Each core
returns a partial [L, D] output-projection contribution; the host sums the
4 partials per batch and adds bo.

v3 design:
  - the host uploads x^T (and all weights) pre-transposed and pre-cast to
    bf16, so the kernel has no PE transposes and no PSUM->SBUF cast copies
  - P (=exp scores) and V stay f32r: the scalar-engine exp writes 4-byte
    output ~20% faster than bf16, and f32r moving operands >=256 wide run
    the PE at full rate
  - phase order: K^T for all four 512-query windows, V for all windows,
    Q^T(w3), then attention windows 3,2,1,0; Q^T(w2..w0) and the previous
    window's output projection run as PE filler inside the ACT-bound
    attention loops, and the kernel tail is the smallest window
  - attention inner loop is software-pipelined one k-tile ahead (S(kt+1)
    issues before PV(kt)) so exp latency never stalls the PE
  - causal diagonal blocks restrict matmul/exp ranges to the valid part
    and affine_select only touches the 128-wide diagonal strip
  - softmax normalize: denominator rows hop to partitions 0/1, one
    reciprocal_approx_fast, then a [2,128]-pattern matmul broadcasts both
    reciprocal rows across 128 partitions in PSUM (no gpsimd
    partition_broadcast), and two DVE multiplies scale O'^T
  - output-projection tiles DMA to DRAM straight out of PSUM (no staging
    copy)
"""

import numpy as np

import concourse.bass as bass
import concourse.tile as tile
from concourse import bacc, mybir
from concourse.bass_utils import run_bass_kernel_spmd
from concourse.vector_clock import VectorClock, ScopedClock

F32 = mybir.dt.float32
F32R = mybir.dt.float32r
BF16 = mybir.dt.bfloat16

B, L, D, H = 2, 2048, 1024, 16
DKH = 64          # head dim
HC = 4            # heads per core
DKC = HC * DKH    # 256 projected cols per core
LW = 512          # query window
NW = L // LW      # 4 windows
NKT = L // 128    # 16 k tiles


class _SplitDrainTileContext(tile.TileContext):
    """The walrus build in this container only supports a single sync-wait
    per Drain instruction; split the kernel-tail drain into one drain per
    outstanding semaphore."""

    def _drain_and_barrier(self, tick_clock, wait_clock):
        gc = tick_clock.global_clock
        n = len(gc)
        active = [i for i in range(n) if gc[i] > 0]
        for i in active:
            vc = VectorClock([gc[j] if j == i else 0 for j in range(n)])
            di = self.nc.sync.drain()
            wait_clock.add_sem_waits(di.ins, ScopedClock({None: vc}))
        self.nc.all_engine_barrier()
        popped = self.nc._tile_sem_poison_stack.pop()
        assert popped is self._sem_poison
        self.nc.clear_and_free_semaphores(list(self.sems.allocated().values()))
        self.nc.all_engine_barrier()


def _ms(ap):
    """memset-safe view: walrus rejects f32r memsets."""
    return ap.bitcast(F32) if ap.dtype == F32R else ap


def build_program() -> bass.Bass:
    nc = bacc.Bacc("TRN2", target_bir_lowering=False, debug=False)

    # x^T streams and weights arrive pre-transposed / pre-cast on the host
    xt_ctx = nc.declare_dram_parameter("xt_ctx", [D, L], BF16, isOutput=False)
    xt_val = nc.declare_dram_parameter("xt_val", [D, L], BF16, isOutput=False)
    wq = nc.declare_dram_parameter("wq", [D, DKC], BF16, isOutput=False)
    wk = nc.declare_dram_parameter("wk", [D, DKC], BF16, isOutput=False)
    wv = nc.declare_dram_parameter("wv", [D, DKC], BF16, isOutput=False)
    bq = nc.declare_dram_parameter("bq", [DKC], F32, isOutput=False)
    bk = nc.declare_dram_parameter("bk", [DKC], F32, isOutput=False)
    bv = nc.declare_dram_parameter("bv", [DKC], F32, isOutput=False)
    wo = nc.declare_dram_parameter("wo", [DKC, D], BF16, isOutput=False)
    pat2_in = nc.declare_dram_parameter("pat2", [2, 128], BF16, isOutput=False)
    out = nc.declare_dram_parameter("out", [L, D], F32, isOutput=True)

    with _SplitDrainTileContext(nc) as tc:
        with (
            tc.tile_pool(name="consts", bufs=1) as consts,
            tc.tile_pool(name="resident", bufs=1) as resident,
            tc.tile_pool(name="pp", bufs=3) as p_pool,
            tc.tile_pool(name="norm", bufs=2) as norm_pool,
            tc.tile_pool(name="evac", bufs=4) as evac_pool,
            tc.tile_pool(name="ps_mm", bufs=2, space="PSUM") as ps_mm,
            tc.tile_pool(name="ps_s", bufs=2, space="PSUM") as ps_s,
            tc.tile_pool(name="ps_o", bufs=2, space="PSUM") as ps_o,
        ):
            # dummy activation: pulls the ~2.7us exp table load off the
            # first attention chunk's critical path
            warm = consts.tile([1, 8], F32, tag="warm")
            nc.vector.memset(warm[:, :], 0.0)
            nc.scalar.activation(warm[:, :], warm[:, :],
                                 func=mybir.ActivationFunctionType.Exp)
            # [2,128] 0/1 pattern: matmul(pat2, r[2,512]) broadcasts r row 0
            # to PSUM partitions 0..63 and row 1 to partitions 64..127.
            # Loaded from DRAM: engines can't write at partition offset 1.
            pat2 = consts.tile([2, 128], BF16, tag="pat2")
            nc.sync.dma_start(out=pat2[:, :], in_=pat2_in[:, :])

            # ---- weights + per-window x^T loads, ordered so K(w0) can
            # start after ~1MB ----
            wk_sb = consts.tile([128, 8, DKC], BF16, tag="wk")
            nc.sync.dma_start(out=wk_sb[:, :, :], in_=wk[:, :].rearrange("(k p) n -> p k n", p=128))
            bk_sb = consts.tile([128, 2], F32, tag="bk")
            nc.sync.dma_start(out=bk_sb[:, :], in_=bk[:].rearrange("(m p) -> p m", p=128))

            xc_sb = resident.tile([128, 8, L], BF16, tag="xc")
            xv_sb = resident.tile([128, 8, L], BF16, tag="xv")

            def load_x(dst, src, lw_, split=1):
                lsl_ = slice(lw_ * LW, (lw_ + 1) * LW)
                for h in range(split):
                    ks = slice(h * 8 // split, (h + 1) * 8 // split)
                    nc.sync.dma_start(
                        out=dst[:, ks, lsl_],
                        in_=src[:, lsl_].rearrange("(k p) l -> p k l", p=128)[:, ks, :])

            load_x(xc_sb, xt_ctx, 0, split=2)

            wq_sb = consts.tile([128, 8, DKC], BF16, tag="wq")
            nc.sync.dma_start(out=wq_sb[:, :, :], in_=wq[:, :].rearrange("(k p) n -> p k n", p=128))
            bq_sb = consts.tile([128, 2], F32, tag="bq")
            nc.sync.dma_start(out=bq_sb[:, :], in_=bq[:].rearrange("(m p) -> p m", p=128))

            for lw_ in range(1, NW):
                load_x(xc_sb, xt_ctx, lw_)

            wv_sb = consts.tile([128, 8, DKC], BF16, tag="wv")
            nc.sync.dma_start(out=wv_sb[:, :, :], in_=wv[:, :].rearrange("(k p) n -> p k n", p=128))
            # bv replicated to all 128 partitions so the V-projection bias
            # folds into the PSUM->SBUF copy on the DVE
            bv_bc = consts.tile([128, DKC], F32, tag="bv")
            nc.sync.dma_start(
                out=bv_bc[:, :], in_=bv[:].unsqueeze(0).broadcast_to([128, DKC]))

            for lw_ in range(NW):
                load_x(xv_sb, xt_val, lw_)

            wo_sb = consts.tile([128, 2, D], BF16, tag="wo")
            nc.sync.dma_start(out=wo_sb[:, :, :], in_=wo[:, :].rearrange("(m p) n -> p m n", p=128))

            # ---- resident projections ----
            qT_sb = resident.tile([128, 2, L], BF16, tag="qT")      # Q^T, head h at [(h%2)*64:+64, h//2, :]
            kT_sb = resident.tile([128, 2, L], BF16, tag="kT")      # K^T, same layout
            v_sb = resident.tile([128, NKT, HC, 1 + DKH], F32R, tag="v")  # V per l-tile/head: [V | ones]
            oT_sb = resident.tile([128, 2, L], BF16, tag="oT")      # normalized O^T, same layout as kT

            nc.vector.memset(_ms(v_sb[:, :, :, DKH:DKH + 1]), 1.0)  # ones col -> denominator row

            def proj_qk(dst, w_sb, b_sb, lw_, m):
                lsl_ = slice(lw_ * LW, (lw_ + 1) * LW)
                pq = ps_mm.tile([128, LW], F32, tag="mm512")
                for k in range(8):
                    nc.tensor.matmul(
                        pq[:, :],
                        w_sb[:, k, m * 128:(m + 1) * 128],
                        xc_sb[:, k, lsl_],
                        start=(k == 0), stop=(k == 7),
                    )
                nc.vector.tensor_scalar_add(dst[:, m, lsl_], pq[:, :], b_sb[:, m:m + 1])

            def proj_v(lw_, a):
                pv = ps_mm.tile([128, LW], F32, tag="mm512")
                for k in range(8):
                    nc.tensor.matmul(
                        pv[:, 0:DKC],
                        xv_sb[:, k, lw_ * LW + a * 128:lw_ * LW + (a + 1) * 128],
                        wv_sb[:, k, :],
                        start=(k == 0), stop=(k == 7),
                    )
                nc.vector.tensor_add(
                    v_sb[:, lw_ * 4 + a, :, 0:DKH],
                    pv[:, 0:DKC].rearrange("p (h d) -> p h d", h=HC),
                    bv_bc[:, :].rearrange("p (h d) -> p h d", h=HC),
                )

            # ---- Phase A: K^T all windows, V all windows, Q^T(w3) ----
            for lw in range(NW):
                for m in range(2):
                    proj_qk(kT_sb, wk_sb, bk_sb, lw, m)
            for lw in range(NW):
                for a in range(4):
                    proj_v(lw, a)
            for m in range(2):
                proj_qk(qT_sb, wq_sb, bq_sb, NW - 1, m)

            # ---- attention, windows largest-first so the tail is small ----
            pend = {}

            def finish_pair(lw_, onorm2, hp, ostg_e, ostg_o, rq):
                """Deferred half of one head pair's normalize: approx
                reciprocal, PE pattern-broadcast, DVE multiplies into O^T."""
                lsl_ = slice(lw_ * LW, (lw_ + 1) * LW)
                rqr = norm_pool.tile([2, LW], F32, tag="rqr")
                nc.vector.reciprocal_approx_fast(rqr[:, :], rq[:, :])
                rqr_b = norm_pool.tile([2, LW], BF16, tag="rqrb")
                nc.vector.tensor_copy(rqr_b[:, :], rqr[:, :])
                rb2 = ps_mm.tile([128, LW], F32, tag="mm512")
                nc.tensor.matmul(rb2[:, :], pat2[:, :], rqr_b[:, :],
                                 start=True, stop=True)
                nc.vector.tensor_mul(
                    oT_sb[0:64, hp, lsl_], ostg_e[0:DKH, :], rb2[0:64, :])
                nc.vector.tensor_mul(
                    onorm2[:, hp, :], ostg_o[0:DKH, :], rb2[64:128, :])

            def finish_normalize(lw_):
                lsl_ = slice(lw_ * LW, (lw_ + 1) * LW)
                onorm2 = norm_pool.tile([DKH, 2, LW], BF16, tag="onorm")
                for hp, ostg_e, ostg_o, rq in pend.pop(lw_):
                    finish_pair(lw_, onorm2, hp, ostg_e, ostg_o, rq)
                nc.gpsimd.dma_start(out=oT_sb[64:128, 0:2, lsl_], in_=onorm2[:, :, :])

            def out_proj_unit(lt, n):
                """One [128, 512] tile of partial = O^T.T @ Wo; the PSUM
                evacuation alternates between DVE and the scalar engine
                (copy lives in every act table, so no table reload)."""
                pop = ps_mm.tile([128, LW], F32, tag="mm512")
                for m in range(2):
                    nc.tensor.matmul(
                        pop[:, :],
                        oT_sb[:, m, lt * 128:(lt + 1) * 128],
                        wo_sb[:, m, n * 512:(n + 1) * 512],
                        start=(m == 0), stop=(m == 1),
                    )
                ost = p_pool.tile([128, LW], F32, tag="ostage")
                if (lt + n) % 2 == 0:
                    nc.vector.tensor_copy(ost[:, :], pop[:, :])
                else:
                    nc.scalar.copy(ost[:, :], pop[:, :])
                nc.sync.dma_start(
                    out=out[lt * 128:(lt + 1) * 128, n * 512:(n + 1) * 512],
                    in_=ost[:, :],
                )

            order = list(range(NW - 1, -1, -1))  # 3, 2, 1, 0
            for wi, lw in enumerate(order):
                lsl = slice(lw * LW, (lw + 1) * LW)
                nkt = 4 * (lw + 1)
                prev = order[wi - 1] if wi > 0 else None
                if prev is not None:
                    finish_normalize(prev)
                # PE filler inside this ACT-bound window: the deferred Q^T
                # projections during w3, the previous window's output
                # projection afterwards
                if wi == 0:
                    filler = [lambda lw_=lw2, m_=m: proj_qk(qT_sb, wq_sb, bq_sb, lw_, m_)
                              for lw2 in range(NW - 2, -1, -1) for m in range(2)]
                    gate = 2   # spread across the window; no oT dependency
                else:
                    filler = [lambda lt_=lt, n_=n: out_proj_unit(lt_, n_)
                              for lt in range(prev * 4, prev * 4 + 4) for n in range(2)]
                    gate = nkt - 4  # wait for finish_normalize to land
                fi = 0
                for hp in range(2):  # head pairs (2hp, 2hp+1)
                    po_e = ps_o.tile([1 + DKH, LW], F32, tag="o")
                    po_o = ps_o.tile([1 + DKH, LW], F32, tag="o")
                    psbs = {}

                    def emit_S(kt):
                        s = kt - 4 * lw
                        qr = slice(128 * s, LW) if s > 0 else slice(0, LW)
                        ksb = ps_s.tile([128, 2, LW], F32, tag="s")
                        nc.tensor.matmul(
                            ksb[:, 0, qr],
                            kT_sb[0:64, hp, kt * 128:(kt + 1) * 128],
                            qT_sb[0:64, hp, lw * LW + qr.start:lw * LW + LW],
                            start=True, stop=True,
                        )
                        nc.tensor.matmul(
                            ksb[:, 1, qr],
                            kT_sb[64:128, hp, kt * 128:(kt + 1) * 128],
                            qT_sb[64:128, hp, lw * LW + qr.start:lw * LW + LW],
                            start=True, stop=True,
                        )
                        psb = p_pool.tile([128, 2, LW], F32R, tag="p")
                        nc.scalar.activation(
                            psb[:, :, qr], ksb[:, :, qr],
                            func=mybir.ActivationFunctionType.Exp,
                            scale=1.0 / np.sqrt(DKH),
                        )
                        if s >= 0:  # diagonal block: zero q<k in its 128-col strip
                            qs = slice(128 * s, 128 * s + 128)
                            for i in range(2):
                                nc.gpsimd.affine_select(
                                    out=psb[:, i, qs], in_=psb[:, i, qs],
                                    compare_op=mybir.AluOpType.is_ge,
                                    fill=0.0, base=0,
                                    pattern=[[1, 128]], channel_multiplier=-1,
                                )
                        psbs[kt] = psb

                    emit_S(0)
                    for kt in range(nkt):
                        if kt + 1 < nkt:
                            emit_S(kt + 1)
                        s = kt - 4 * lw
                        qr = slice(128 * s, LW) if s > 0 else slice(0, LW)
                        psb = psbs.pop(kt)
                        nc.tensor.matmul(
                            po_e[:, qr], v_sb[:, kt, 2 * hp, :], psb[:, 0, qr],
                            start=(kt == 0), stop=(kt == nkt - 1),
                        )
                        nc.tensor.matmul(
                            po_o[:, qr], v_sb[:, kt, 2 * hp + 1, :], psb[:, 1, qr],
                            start=(kt == 0), stop=(kt == nkt - 1),
                        )
                        if fi < len(filler) and kt >= gate:
                            filler[fi]()
                            fi += 1
                    # evacuate both PSUM banks right away; the denominator
                    # rows hop onto partitions 0/1 of one tile; the
                    # reciprocal + multiplies are deferred a window so they
                    # never precede the next window's PSUM->SBUF copies in
                    # the DVE's in-order stream
                    ostg_e = evac_pool.tile([1 + DKH, LW], F32, tag="ostg")
                    nc.vector.tensor_copy(ostg_e[:, :], po_e[:, :])
                    ostg_o = evac_pool.tile([1 + DKH, LW], F32, tag="ostg")
                    nc.vector.tensor_copy(ostg_o[:, :], po_o[:, :])
                    rq = norm_pool.tile([2, LW], F32, tag="rq")
                    nc.gpsimd.dma_start(out=rq[0:1, :], in_=ostg_e[64:65, :])
                    nc.gpsimd.dma_start(out=rq[1:2, :], in_=ostg_o[64:65, :])
                    if wi == NW - 1:
                        # last processed window: finish right away so hp0's
                        # chain overlaps hp1's attention chunks
                        if hp == 0:
                            last_onorm2 = norm_pool.tile([DKH, 2, LW], BF16, tag="onorm")
                        finish_pair(lw, last_onorm2, hp, ostg_e, ostg_o, rq)
                        if hp == 1:
                            nc.gpsimd.dma_start(
                                out=oT_sb[64:128, 0:2, lsl], in_=last_onorm2[:, :, :])
                    else:
                        pend.setdefault(lw, []).append((hp, ostg_e, ostg_o, rq))
                while fi < len(filler):
                    filler[fi]()
                    fi += 1

            last = order[-1]
            for lt in range(last * 4, last * 4 + 4):
                for n in range(2):
                    out_proj_unit(lt, n)

    nc.compile()
    return nc


_CACHE = {}


def _program() -> bass.Bass:
    if "nc" not in _CACHE:
        _CACHE["nc"] = build_program()
    return _CACHE["nc"]


def make_in_maps(inputs):
    import ml_dtypes
    bf16 = ml_dtypes.bfloat16
    ctx = np.asarray(inputs["context_sequence"], np.float32)
    val = np.asarray(inputs["value_sequence"], np.float32)
    Wq = np.asarray(inputs["Wq"], np.float32)
    Wk = np.asarray(inputs["Wk"], np.float32)
    Wv = np.asarray(inputs["Wv"], np.float32)
    Wo = np.asarray(inputs["Wo"], np.float32)
    bq = np.asarray(inputs["bq"], np.float32)
    bk = np.asarray(inputs["bk"], np.float32)
    bv = np.asarray(inputs["bv"], np.float32)
    pat2 = np.zeros((2, 128), np.float32)
    pat2[0, 0:64] = 1.0
    pat2[1, 64:128] = 1.0
    pat2 = pat2.astype(bf16)
    xt_ctx = [np.ascontiguousarray(ctx[b].T).astype(bf16) for b in range(B)]
    xt_val = [np.ascontiguousarray(val[b].T).astype(bf16) for b in range(B)]
    in_maps = []
    for c in range(8):
        b, hg = divmod(c, 4)
        cols = slice(hg * DKC, (hg + 1) * DKC)
        in_maps.append({
            "xt_ctx": xt_ctx[b],
            "xt_val": xt_val[b],
            "wq": np.ascontiguousarray(Wq[:, cols]).astype(bf16),
            "wk": np.ascontiguousarray(Wk[:, cols]).astype(bf16),
            "wv": np.ascontiguousarray(Wv[:, cols]).astype(bf16),
            "bq": np.ascontiguousarray(bq[cols]),
            "bk": np.ascontiguousarray(bk[cols]),
            "bv": np.ascontiguousarray(bv[cols]),
            "wo": np.ascontiguousarray(Wo[cols, :]).astype(bf16),
            "pat2": pat2,
        })
    return in_maps


def combine_outputs(results, bo):
    bo = np.asarray(bo, np.float32)
    outs = [np.asarray(r["out"], np.float32) for r in results]
    full = np.empty((B, L, D), np.float32)
    for b in range(B):
        acc = np.zeros((L, D), np.float64)
        for c in range(4 * b, 4 * b + 4):
            acc += outs[c]
        full[b] = (acc + bo).astype(np.float32)
    return full


def _numpy_fallback(inputs):
    """Reference semantics for a non-causal mask (the TRN kernel hardcodes
    the causal structure)."""
    ctx = np.asarray(inputs["context_sequence"], np.float32)
    val = np.asarray(inputs["value_sequence"], np.float32)
    mask = np.asarray(inputs["mask"]) != 0
    Q = (ctx @ inputs["Wq"] + inputs["bq"]).reshape(B, L, H, DKH)
    Kp = (ctx @ inputs["Wk"] + inputs["bk"]).reshape(B, L, H, DKH)
    V = (val @ inputs["Wv"] + inputs["bv"]).reshape(B, L, H, DKH)
    outs = np.zeros((B, L, D), np.float32)
    for b in range(B):
        for h in range(H):
            s = (Q[b, :, h, :] @ Kp[b, :, h, :].T) / np.sqrt(np.float32(DKH))
            s = np.where(mask, s, -np.inf)
            s = s - s.max(axis=1, keepdims=True)
            p = np.exp(s)
            p /= p.sum(axis=1, keepdims=True)
            outs[b] += (p @ V[b, :, h, :]) @ np.asarray(inputs["Wo"])[h * DKH:(h + 1) * DKH, :]
    return outs + np.asarray(inputs["bo"], np.float32)


def kernel(**inputs) -> np.ndarray:
    mask = np.asarray(inputs["mask"])
    if not np.array_equal(mask != 0, np.tril(np.ones((L, L), bool))):
        return _numpy_fallback(inputs)
    nc = _program()
    in_maps = make_in_maps(inputs)
    last_err = None
    for _attempt in range(3):
        try:
            res = run_bass_kernel_spmd(nc, in_maps, list(range(8)))
            break
        except Exception as e:  # transient NRT device wedges clear on retry
            last_err = e
    else:
        raise last_err
    return combine_outputs(res.results, inputs["bo"])


if __name__ == "__main__":
    rng = np.random.default_rng(0)
    demo = {
        "context_sequence": rng.normal(size=(B, L, D)).astype(np.float32),
        "value_sequence": rng.normal(size=(B, L, D)).astype(np.float32),
        "mask": np.tril(np.ones((L, L), np.int32)),
        **{f"W{n}": (rng.normal(size=(D, D)) / 32).astype(np.float32) for n in "qkvo"},
        **{f"b{n}": (rng.normal(size=(D,)) / 32).astype(np.float32) for n in "qkvo"},
    }
    out = kernel(**demo)
    print(out.shape, out.dtype)


# revision 23
# speedup vs baseline: 1.4374x; 1.0287x over previous
"""Multi-head causal attention (B=2, L=2048, D=1024, H=16) on 8 TRN2 cores.

Sharding: data-parallel over batch (cores 0-3 -> b=0, cores 4-7 -> b=1),
tensor-parallel over heads (each core computes 4 of the 16 heads: the
matching 256-column slice of Wq/Wk/Wv and 256-row slice of Wo).  Each core
returns a partial [L, D] output-projection contribution; the host sums the
4 partials per batch and adds bo.

v3 design:
  - the host uploads x^T (and all weights) pre-transposed and pre-cast to
    bf16, so the kernel has no PE transposes and no PSUM->SBUF cast copies
  - P (=exp scores) and V stay f32r: the scalar-engine exp writes 4-byte
    output ~20% faster than bf16, and f32r moving operands >=256 wide run
    the PE at full rate
  - phase order: K^T for all four 512-query windows, V for all windows,
    Q^T(w3), then attention windows 3,2,1,0; Q^T(w2..w0) and the previous
    window's output projection run as PE filler inside the ACT-bound
    attention loops, and the kernel tail is the smallest window
  - attention inner loop is software-pipelined one k-tile ahead (S(kt+1)
    issues before PV(kt)) so exp latency never stalls the PE
  - causal diagonal blocks restrict matmul/exp ranges to the valid part
    and affine_select only touches the 128-wide diagonal strip
  - softmax normalize: denominator rows hop to partitions 0/1, one
    reciprocal_approx_fast, then a [2,128]-pattern matmul broadcasts both
    reciprocal rows across 128 partitions in PSUM (no gpsimd
    partition_broadcast), and two DVE multiplies scale O'^T
  - output-projection tiles DMA to DRAM straight out of PSUM (no staging
    copy)
"""

import numpy as np

import concourse.bass as bass
import concourse.tile as tile
from concourse import bacc, mybir
from concourse.bass_utils import run_bass_kernel_spmd
from concourse.vector_clock import VectorClock, ScopedClock

F32 = mybir.dt.float32
F32R = mybir.dt.float32r
BF16 = mybir.dt.bfloat16

B, L, D, H = 2, 2048, 1024, 16
DKH = 64          # head dim
HC = 4            # heads per core
DKC = HC * DKH    # 256 projected cols per core
LW = 512          # query window
NW = L // LW      # 4 windows
NKT = L // 128    # 16 k tiles


class _SplitDrainTileContext(tile.TileContext):
    """The walrus build in this container only supports a single sync-wait
    per Drain instruction; split the kernel-tail drain into one drain per
    outstanding semaphore."""

    def _drain_and_barrier(self, tick_clock, wait_clock):
        gc = tick_clock.global_clock
        n = len(gc)
        active = [i for i in range(n) if gc[i] > 0]
        for i in active:
            vc = VectorClock([gc[j] if j == i else 0 for j in range(n)])
            di = self.nc.sync.drain()
            wait_clock.add_sem_waits(di.ins, ScopedClock({None: vc}))
        self.nc.all_engine_barrier()
        popped = self.nc._tile_sem_poison_stack.pop()
        assert popped is self._sem_poison
        self.nc.clear_and_free_semaphores(list(self.sems.allocated().values()))
        self.nc.all_engine_barrier()


def _ms(ap):
    """memset-safe view: walrus rejects f32r memsets."""
    return ap.bitcast(F32) if ap.dtype == F32R else ap


def build_program() -> bass.Bass:
    nc = bacc.Bacc("TRN2", target_bir_lowering=False, debug=False)

    # x^T streams and weights arrive pre-transposed / pre-cast on the host
    xt_ctx = nc.declare_dram_parameter("xt_ctx", [D, L], BF16, isOutput=False)
    xt_val = nc.declare_dram_parameter("xt_val", [D, L], BF16, isOutput=False)
    wq = nc.declare_dram_parameter("wq", [D, DKC], BF16, isOutput=False)
    wk = nc.declare_dram_parameter("wk", [D, DKC], BF16, isOutput=False)
    wv = nc.declare_dram_parameter("wv", [D, DKC], BF16, isOutput=False)
    bq = nc.declare_dram_parameter("bq", [DKC], F32, isOutput=False)
    bk = nc.declare_dram_parameter("bk", [DKC], F32, isOutput=False)
    bv = nc.declare_dram_parameter("bv", [DKC], F32, isOutput=False)
    wo = nc.declare_dram_parameter("wo", [DKC, D], BF16, isOutput=False)
    pat2_in = nc.declare_dram_parameter("pat2", [2, 128], BF16, isOutput=False)
    out = nc.declare_dram_parameter("out", [L, D], F32, isOutput=True)

    with _SplitDrainTileContext(nc) as tc:
        with (
            tc.tile_pool(name="consts", bufs=1) as consts,
            tc.tile_pool(name="resident", bufs=1) as resident,
            tc.tile_pool(name="pp", bufs=3) as p_pool,
            tc.tile_pool(name="norm", bufs=2) as norm_pool,
            tc.tile_pool(name="evac", bufs=4) as evac_pool,
            tc.tile_pool(name="ps_mm", bufs=2, space="PSUM") as ps_mm,
            tc.tile_pool(name="ps_s", bufs=2, space="PSUM") as ps_s,
            tc.tile_pool(name="ps_o", bufs=2, space="PSUM") as ps_o,
        ):
            # dummy activation: pulls the ~2.7us exp table load off the
            # first attention chunk's critical path
            warm = consts.tile([1, 8], F32, tag="warm")
            nc.vector.memset(warm[:, :], 0.0)
            nc.scalar.activation(warm[:, :], warm[:, :],
                                 func=mybir.ActivationFunctionType.Exp)
            # [2,128] 0/1 pattern: matmul(pat2, r[2,512]) broadcasts r row 0
            # to PSUM partitions 0..63 and row 1 to partitions 64..127.
            # Loaded from DRAM: engines can't write at partition offset 1.
            pat2 = consts.tile([2, 128], BF16, tag="pat2")
            nc.sync.dma_start(out=pat2[:, :], in_=pat2_in[:, :])

            # ---- weights + per-window x^T loads, ordered so K(w0) can
            # start after ~1MB ----
            wk_sb = consts.tile([128, 8, DKC], BF16, tag="wk")
            nc.sync.dma_start(out=wk_sb[:, :, :], in_=wk[:, :].rearrange("(k p) n -> p k n", p=128))
            bk_sb = consts.tile([128, 2], F32, tag="bk")
            nc.sync.dma_start(out=bk_sb[:, :], in_=bk[:].rearrange("(m p) -> p m", p=128))

            xc_sb = resident.tile([128, 8, L], BF16, tag="xc")
            xv_sb = resident.tile([128, 8, L], BF16, tag="xv")

            def load_x(dst, src, lw_, split=1):
                lsl_ = slice(lw_ * LW, (lw_ + 1) * LW)
                for h in range(split):
                    ks = slice(h * 8 // split, (h + 1) * 8 // split)
                    nc.sync.dma_start(
                        out=dst[:, ks, lsl_],
                        in_=src[:, lsl_].rearrange("(k p) l -> p k l", p=128)[:, ks, :])

            load_x(xc_sb, xt_ctx, 0, split=2)

            wq_sb = consts.tile([128, 8, DKC], BF16, tag="wq")
            nc.sync.dma_start(out=wq_sb[:, :, :], in_=wq[:, :].rearrange("(k p) n -> p k n", p=128))
            bq_sb = consts.tile([128, 2], F32, tag="bq")
            nc.sync.dma_start(out=bq_sb[:, :], in_=bq[:].rearrange("(m p) -> p m", p=128))

            load_x(xc_sb, xt_ctx, NW - 1)  # Q(w3) is the second projection

            wv_sb = consts.tile([128, 8, DKC], BF16, tag="wv")
            nc.sync.dma_start(out=wv_sb[:, :, :], in_=wv[:, :].rearrange("(k p) n -> p k n", p=128))
            # bv replicated to all 128 partitions so the V-projection bias
            # folds into the PSUM->SBUF copy on the DVE
            bv_bc = consts.tile([128, DKC], F32, tag="bv")
            nc.sync.dma_start(
                out=bv_bc[:, :], in_=bv[:].unsqueeze(0).broadcast_to([128, DKC]))

            load_x(xv_sb, xt_val, 0)
            for lw_ in range(1, NW - 1):
                load_x(xc_sb, xt_ctx, lw_)
            for lw_ in range(1, NW):
                load_x(xv_sb, xt_val, lw_)

            wo_sb = consts.tile([128, 2, D], BF16, tag="wo")
            nc.sync.dma_start(out=wo_sb[:, :, :], in_=wo[:, :].rearrange("(m p) n -> p m n", p=128))

            # ---- resident projections ----
            qT_sb = resident.tile([128, 2, L], BF16, tag="qT")      # Q^T, head h at [(h%2)*64:+64, h//2, :]
            kT_sb = resident.tile([128, 2, L], BF16, tag="kT")      # K^T, same layout
            v_sb = resident.tile([128, NKT, HC, 1 + DKH], F32R, tag="v")  # V per l-tile/head: [V | ones]
            oT_sb = resident.tile([128, 2, L], BF16, tag="oT")      # normalized O^T, same layout as kT

            nc.vector.memset(_ms(v_sb[:, :, :, DKH:DKH + 1]), 1.0)  # ones col -> denominator row

            def proj_qk(dst, w_sb, b_sb, lw_, m):
                lsl_ = slice(lw_ * LW, (lw_ + 1) * LW)
                pq = ps_mm.tile([128, LW], F32, tag="mm512")
                for k in range(8):
                    nc.tensor.matmul(
                        pq[:, :],
                        w_sb[:, k, m * 128:(m + 1) * 128],
                        xc_sb[:, k, lsl_],
                        start=(k == 0), stop=(k == 7),
                    )
                nc.vector.tensor_scalar_add(dst[:, m, lsl_], pq[:, :], b_sb[:, m:m + 1])

            def proj_v(lw_, a):
                pv = ps_mm.tile([128, LW], F32, tag="mm512")
                for k in range(8):
                    nc.tensor.matmul(
                        pv[:, 0:DKC],
                        xv_sb[:, k, lw_ * LW + a * 128:lw_ * LW + (a + 1) * 128],
                        wv_sb[:, k, :],
                        start=(k == 0), stop=(k == 7),
                    )
                nc.vector.tensor_add(
                    v_sb[:, lw_ * 4 + a, :, 0:DKH],
                    pv[:, 0:DKC].rearrange("p (h d) -> p h d", h=HC),
                    bv_bc[:, :].rearrange("p (h d) -> p h d", h=HC),
                )

            # ---- Phase A: just enough to start attention(w3) ----
            # K(w0) and Q(w3); every other projection is deadline-scheduled
            # PE filler inside w3's attention.
            for m in range(2):
                proj_qk(kT_sb, wk_sb, bk_sb, 0, m)
            for m in range(2):
                proj_qk(qT_sb, wq_sb, bq_sb, NW - 1, m)

            # ---- attention, windows largest-first so the tail is small ----
            pend = {}

            def finish_pair(lw_, onorm2, hp, ostg_e, ostg_o, rq):
                """Deferred half of one head pair's normalize: approx
                reciprocal, PE pattern-broadcast, DVE multiplies into O^T."""
                lsl_ = slice(lw_ * LW, (lw_ + 1) * LW)
                rqr = norm_pool.tile([2, LW], F32, tag="rqr")
                nc.vector.reciprocal_approx_fast(rqr[:, :], rq[:, :])
                rqr_b = norm_pool.tile([2, LW], BF16, tag="rqrb")
                nc.vector.tensor_copy(rqr_b[:, :], rqr[:, :])
                rb2 = ps_mm.tile([128, LW], F32, tag="mm512")
                nc.tensor.matmul(rb2[:, :], pat2[:, :], rqr_b[:, :],
                                 start=True, stop=True)
                nc.vector.tensor_mul(
                    oT_sb[0:64, hp, lsl_], ostg_e[0:DKH, :], rb2[0:64, :])
                nc.vector.tensor_mul(
                    onorm2[:, hp, :], ostg_o[0:DKH, :], rb2[64:128, :])

            def finish_normalize(lw_):
                lsl_ = slice(lw_ * LW, (lw_ + 1) * LW)
                onorm2 = norm_pool.tile([DKH, 2, LW], BF16, tag="onorm")
                for hp, ostg_e, ostg_o, rq in pend.pop(lw_):
                    finish_pair(lw_, onorm2, hp, ostg_e, ostg_o, rq)
                nc.gpsimd.dma_start(out=oT_sb[64:128, 0:2, lsl_], in_=onorm2[:, :, :])

            def out_proj_unit(lt, n):
                """One [128, 512] tile of partial = O^T.T @ Wo; the PSUM
                evacuation alternates between DVE and the scalar engine
                (copy lives in every act table, so no table reload)."""
                pop = ps_mm.tile([128, LW], F32, tag="mm512")
                for m in range(2):
                    nc.tensor.matmul(
                        pop[:, :],
                        oT_sb[:, m, lt * 128:(lt + 1) * 128],
                        wo_sb[:, m, n * 512:(n + 1) * 512],
                        start=(m == 0), stop=(m == 1),
                    )
                ost = p_pool.tile([128, LW], F32, tag="ostage")
                if (lt + n) % 2 == 0:
                    nc.vector.tensor_copy(ost[:, :], pop[:, :])
                else:
                    nc.scalar.copy(ost[:, :], pop[:, :])
                nc.sync.dma_start(
                    out=out[lt * 128:(lt + 1) * 128, n * 512:(n + 1) * 512],
                    in_=ost[:, :],
                )

            order = list(range(NW - 1, -1, -1))  # 3, 2, 1, 0
            for wi, lw in enumerate(order):
                lsl = slice(lw * LW, (lw + 1) * LW)
                nkt = 4 * (lw + 1)
                prev = order[wi - 1] if wi > 0 else None
                if prev is not None:
                    finish_normalize(prev)
                # PE filler inside this ACT-bound window.  During w3 (the
                # first processed window) the remaining K/V/Q projections
                # stream in with deadlines: K(wa) before S(4a) is issued,
                # V(wa) unit a' before PV(4a+a') consumes it.  Later windows
                # interleave the previous window's output projection.
                if wi == 0:
                    filler = []  # (deadline, emit)
                    for a in range(4):
                        filler.append((a,
                                       lambda a_=a: proj_v(0, a_)))
                    for lw2 in range(1, NW):
                        for m in range(2):
                            filler.append((4 * lw2 - 1,
                                           lambda lw_=lw2, m_=m: proj_qk(kT_sb, wk_sb, bk_sb, lw_, m_)))
                        for a in range(4):
                            filler.append((4 * lw2 + a,
                                           lambda lw_=lw2, a_=a: proj_v(lw_, a_)))
                    for lw2 in range(NW - 2, -1, -1):
                        for m in range(2):
                            filler.append((99,
                                           lambda lw_=lw2, m_=m: proj_qk(qT_sb, wq_sb, bq_sb, lw_, m_)))
                    gate = 0
                else:
                    filler = [(99, lambda lt_=lt, n_=n: out_proj_unit(lt_, n_))
                              for lt in range(prev * 4, prev * 4 + 4) for n in range(2)]
                    gate = max(2, nkt - 6)
                fi = 0
                for hp in range(2):  # head pairs (2hp, 2hp+1)
                    po_e = ps_o.tile([1 + DKH, LW], F32, tag="o")
                    po_o = ps_o.tile([1 + DKH, LW], F32, tag="o")
                    psbs = {}

                    def emit_S(kt):
                        s = kt - 4 * lw
                        qr = slice(128 * s, LW) if s > 0 else slice(0, LW)
                        ksb = ps_s.tile([128, 2, LW], F32, tag="s")
                        nc.tensor.matmul(
                            ksb[:, 0, qr],
                            kT_sb[0:64, hp, kt * 128:(kt + 1) * 128],
                            qT_sb[0:64, hp, lw * LW + qr.start:lw * LW + LW],
                            start=True, stop=True,
                        )
                        nc.tensor.matmul(
                            ksb[:, 1, qr],
                            kT_sb[64:128, hp, kt * 128:(kt + 1) * 128],
                            qT_sb[64:128, hp, lw * LW + qr.start:lw * LW + LW],
                            start=True, stop=True,
                        )
                        psb = p_pool.tile([128, 2, LW], F32R, tag="p")
                        nc.scalar.activation(
                            psb[:, :, qr], ksb[:, :, qr],
                            func=mybir.ActivationFunctionType.Exp,
                            scale=1.0 / np.sqrt(DKH),
                        )
                        if s >= 0:  # diagonal block: zero q<k in its 128-col strip
                            qs = slice(128 * s, 128 * s + 128)
                            for i in range(2):
                                nc.gpsimd.affine_select(
                                    out=psb[:, i, qs], in_=psb[:, i, qs],
                                    compare_op=mybir.AluOpType.is_ge,
                                    fill=0.0, base=0,
                                    pattern=[[1, 128]], channel_multiplier=-1,
                                )
                        psbs[kt] = psb

                    emit_S(0)
                    for kt in range(nkt):
                        # flush fillers: hard deadlines (hp0 of w3) always;
                        # otherwise paced at `cap` per iteration after `gate`
                        cap = 1 if wi == 0 else 2
                        flushed = 0
                        while fi < len(filler):
                            dl = filler[fi][0]
                            if (hp == 0 and dl <= kt) or (flushed < cap and kt >= gate):
                                filler[fi][1]()
                                fi += 1
                                flushed += 1
                            else:
                                break
                        if kt + 1 < nkt:
                            emit_S(kt + 1)
                        s = kt - 4 * lw
                        qr = slice(128 * s, LW) if s > 0 else slice(0, LW)
                        psb = psbs.pop(kt)
                        nc.tensor.matmul(
                            po_e[:, qr], v_sb[:, kt, 2 * hp, :], psb[:, 0, qr],
                            start=(kt == 0), stop=(kt == nkt - 1),
                        )
                        nc.tensor.matmul(
                            po_o[:, qr], v_sb[:, kt, 2 * hp + 1, :], psb[:, 1, qr],
                            start=(kt == 0), stop=(kt == nkt - 1),
                        )
                    # evacuate both PSUM banks right away; the denominator
                    # rows hop onto partitions 0/1 of one tile; the
                    # reciprocal + multiplies are deferred a window so they
                    # never precede the next window's PSUM->SBUF copies in
                    # the DVE's in-order stream
                    ostg_e = evac_pool.tile([1 + DKH, LW], F32, tag="ostg")
                    nc.vector.tensor_copy(ostg_e[:, :], po_e[:, :])
                    ostg_o = evac_pool.tile([1 + DKH, LW], F32, tag="ostg")
                    nc.vector.tensor_copy(ostg_o[:, :], po_o[:, :])
                    rq = norm_pool.tile([2, LW], F32, tag="rq")
                    nc.gpsimd.dma_start(out=rq[0:1, :], in_=ostg_e[64:65, :])
                    nc.gpsimd.dma_start(out=rq[1:2, :], in_=ostg_o[64:65, :])
                    if wi == NW - 1:
                        # last processed window: finish right away so hp0's
                        # chain overlaps hp1's attention chunks
                        if hp == 0:
                            last_onorm2 = norm_pool.tile([DKH, 2, LW], BF16, tag="onorm")
                        finish_pair(lw, last_onorm2, hp, ostg_e, ostg_o, rq)
                        if hp == 1:
                            nc.gpsimd.dma_start(
                                out=oT_sb[64:128, 0:2, lsl], in_=last_onorm2[:, :, :])
                    else:
                        pend.setdefault(lw, []).append((hp, ostg_e, ostg_o, rq))
                while fi < len(filler):
                    filler[fi][1]()
                    fi += 1

            last = order[-1]
            for lt in range(last * 4, last * 4 + 4):
                for n in range(2):
                    out_proj_unit(lt, n)

    nc.compile()
    return nc


_CACHE = {}


def _program() -> bass.Bass:
    if "nc" not in _CACHE:
        _CACHE["nc"] = build_program()
    return _CACHE["nc"]


def make_in_maps(inputs):
    import ml_dtypes
    bf16 = ml_dtypes.bfloat16
    ctx = np.asarray(inputs["context_sequence"], np.float32)
    val = np.asarray(inputs["value_sequence"], np.float32)
    Wq = np.asarray(inputs["Wq"], np.float32)
    Wk = np.asarray(inputs["Wk"], np.float32)
    Wv = np.asarray(inputs["Wv"], np.float32)
    Wo = np.asarray(inputs["Wo"], np.float32)
    bq = np.asarray(inputs["bq"], np.float32)
    bk = np.asarray(inputs["bk"], np.float32)
    bv = np.asarray(inputs["bv"], np.float32)
    pat2 = np.zeros((2, 128), np.float32)
    pat2[0, 0:64] = 1.0
    pat2[1, 64:128] = 1.0
    pat2 = pat2.astype(bf16)
    xt_ctx = [np.ascontiguousarray(ctx[b].T).astype(bf16) for b in range(B)]
    xt_val = [np.ascontiguousarray(val[b].T).astype(bf16) for b in range(B)]
    in_maps = []
    for c in range(8):
        b, hg = divmod(c, 4)
        cols = slice(hg * DKC, (hg + 1) * DKC)
        in_maps.append({
            "xt_ctx": xt_ctx[b],
            "xt_val": xt_val[b],
            "wq": np.ascontiguousarray(Wq[:, cols]).astype(bf16),
            "wk": np.ascontiguousarray(Wk[:, cols]).astype(bf16),
            "wv": np.ascontiguousarray(Wv[:, cols]).astype(bf16),
            "bq": np.ascontiguousarray(bq[cols]),
            "bk": np.ascontiguousarray(bk[cols]),
            "bv": np.ascontiguousarray(bv[cols]),
            "wo": np.ascontiguousarray(Wo[cols, :]).astype(bf16),
            "pat2": pat2,
        })
    return in_maps


def combine_outputs(results, bo):
    bo = np.asarray(bo, np.float32)
    outs = [np.asarray(r["out"], np.float32) for r in results]
    full = np.empty((B, L, D), np.float32)
    for b in range(B):
        acc = np.zeros((L, D), np.float64)
        for c in range(4 * b, 4 * b + 4):
            acc += outs[c]
        full[b] = (acc + bo).astype(np.float32)
    return full


def _numpy_fallback(inputs):
    """Reference semantics for a non-causal mask (the TRN kernel hardcodes
    the causal structure)."""
    ctx = np.asarray(inputs["context_sequence"], np.float32)
    val = np.asarray(inputs["value_sequence"], np.float32)
    mask = np.asarray(inputs["mask"]) != 0
    Q = (ctx @ inputs["Wq"] + inputs["bq"]).reshape(B, L, H, DKH)
    Kp = (ctx @ inputs["Wk"] + inputs["bk"]).reshape(B, L, H, DKH)
    V = (val @ inputs["Wv"] + inputs["bv"]).reshape(B, L, H, DKH)
    outs = np.zeros((B, L, D), np.float32)
    for b in range(B):
        for h in range(H):
            s = (Q[b, :, h, :] @ Kp[b, :, h, :].T) / np.sqrt(np.float32(DKH))
            s = np.where(mask, s, -np.inf)
            s = s - s.max(axis=1, keepdims=True)
            p = np.exp(s)
            p /= p.sum(axis=1, keepdims=True)
            outs[b] += (p @ V[b, :, h, :]) @ np.asarray(inputs["Wo"])[h * DKH:(h + 1) * DKH, :]
    return outs + np.asarray(inputs["bo"], np.float32)


def kernel(**inputs) -> np.ndarray:
    mask = np.asarray(inputs["mask"])
    if not np.array_equal(mask != 0, np.tril(np.ones((L, L), bool))):
        return _numpy_fallback(inputs)
    nc = _program()
    in_maps = make_in_maps(inputs)
    last_err = None
    for _attempt in range(3):
        try:
            res = run_bass_kernel_spmd(nc, in_maps, list(range(8)))
            break
        except Exception as e:  # transient NRT device wedges clear on retry
            last_err = e
    else:
        raise last_err
    return combine_outputs(res.results, inputs["bo"])


if __name__ == "__main__":
    rng = np.random.default_rng(0)
    demo = {
        "context_sequence": rng.normal(size=(B, L, D)).astype(np.float32),
        "value_sequence": rng.normal(size=(B, L, D)).astype(np.float32),
        "mask": np.tril(np.ones((L, L), np.int32)),
        **{f"W{n}": (rng.normal(size=(D, D)) / 32).astype(np.float32) for n in "qkvo"},
        **{f"b{n}": (rng.normal(size=(D,)) / 32).astype(np.float32) for n in "qkvo"},
    }
    out = kernel(**demo)
    print(out.shape, out.dtype)
